# revision 16
# baseline (speedup 1.0000x reference)
"""CTC loss (nn_CTCLoss) Trainium2 Bass kernel.

Sharding: data-parallel over batch N across 8 cores (8 samples/core).

Per core:
  * Bulk pass: stream the (T, NL=8, C) f32 shard through SBUF as
    [128=(n,t16), C] tiles.  ScalarE computes exp() with a fused per-row
    accumulate (softmax denominator per (t, n)); GpSimd ap_gather pulls
    the S=2L+1 extended-label columns per sample (index lists are
    host-prepared; each 16-partition group shares one sample's list);
    ScalarE exponentiates the gathered logits (bias=+SHIFT).  q rows plus
    the accumulator column bounce through a DRAM scratch to move t from
    partitions onto the free axis in DP order (the backward direction is
    stored time-reversed and gathered state-reversed, so both chains read
    forward).
  * DP pass on VectorE in the probability domain: forward chain on
    partitions 0-7, time/state-reversed backward chain on partitions
    8-15, advanced together by 3 (no label repeats) or 4 (general)
    instructions per time step; T/2 sequential steps per chain.  The
    softmax denominator is folded in per step via the scalar slot of
    scalar_tensor_tensor (scalar = 1/acc, periodically also carrying a
    renormalisation factor).
  * Final alpha/beta states plus renorm maxima (16 x ~70 floats/core) go
    back to the host, which stitches the chains at the midpoint in
    float64 and takes the batch mean.
"""

import sys

import numpy as np

for _p in ("/root/.axon_site", "/root/.axon_site/_ro/trn_rl_repo", "/opt/trn_rl_repo"):
    if _p not in sys.path:
        sys.path.append(_p)

NCORES = 8
NL = 8                   # samples per core
TG = 16                  # time steps per tile group (128 = NL * TG partitions)
BLANK = 0
PAD = 2                  # leading zero pad columns in DP tiles

# problem dims (the graded configuration)
T, N, C, L = 256, 64, 4096, 32


def _derived(T_, C_, L_, use_renorm=True):
    S_ = 2 * L_ + 1
    GW_ = S_ + 1                       # q row + acc column in DRAM scratch
    NIDX_ = ((S_ + 15) // 16) * 16     # gather list length (%16 == 0)
    NPAIR_ = T_ // (2 * TG)
    TH_ = T_ // 2                      # steps per chain
    if use_renorm:
        SHIFT_ = float(np.log(C_) + 0.5)
        renorm_ = tuple(range(31, TH_ - 1, 32))
    else:
        # -1.0 cancels the ~e^1 per-step growth of the summed mass; the
        # chain then stays inside fp32 range with no renorms at all
        SHIFT_ = float(np.log(C_) + 0.5) - 1.0
        renorm_ = ()
    return S_, GW_, NIDX_, NPAIR_, TH_, SHIFT_, renorm_


# ----------------------------------------------------------------------------
# host-side helpers
# ----------------------------------------------------------------------------

def _ext_labels(t2d, S_):
    ext = np.zeros((t2d.shape[0], S_), np.int64)
    ext[:, 1::2] = t2d
    return ext


def _skip_mask(ext):
    sidx = np.arange(ext.shape[1])
    return (
        (sidx[None, :] >= 2)
        & (ext != BLANK)
        & (ext != np.roll(ext, 2, axis=1))
    )


def _ref_numpy(preds, t2d, pred_lengths, target_lengths):
    """float64 port of the reference (fallback path)."""
    preds = preds.astype(np.float64)
    Tn, n = preds.shape[0], preds.shape[1]
    S_ = 2 * t2d.shape[1] + 1
    mx = preds.max(axis=2, keepdims=True)
    lp = preds - mx - np.log(np.exp(preds - mx).sum(axis=2, keepdims=True))
    ext = _ext_labels(t2d, S_)
    lpe = lp[:, np.arange(n)[:, None], ext]
    skip_ok = _skip_mask(ext)
    NEGI = -1e30
    sidx = np.arange(S_)
    valid = sidx[None, :] < (2 * target_lengths[:, None] + 1)
    alpha = np.full((n, S_), NEGI)
    alpha[:, 0] = lpe[0, :, 0]
    alpha[:, 1] = np.where(target_lengths > 0, lpe[0, :, 1], NEGI)
    alpha = np.where(valid, alpha, NEGI)

    def lse(*a):
        m = np.maximum.reduce(a)
        m = np.where(np.isfinite(m), m, 0.0)
        return m + np.log(sum(np.exp(x - m) for x in a))

    for t in range(1, Tn):
        a2 = np.concatenate([np.full((n, 1), NEGI), alpha[:, :-1]], 1)
        a3 = np.concatenate([np.full((n, 2), NEGI), alpha[:, :-2]], 1)
        a3 = np.where(skip_ok, a3, NEGI)
        new = np.where(valid, lse(alpha, a2, a3) + lpe[t], NEGI)
        active = (t < pred_lengths)[:, None]
        alpha = np.where(active, new, alpha)
    end = 2 * target_lengths
    a_last = alpha[np.arange(n), end]
    a_prev = alpha[np.arange(n), np.maximum(end - 1, 0)]
    a_prev = np.where(target_lengths > 0, a_prev, NEGI)
    nll = -lse(a_last, a_prev)
    nll = np.where(np.isfinite(nll) & (nll < 1e29), nll, 0.0)
    return np.float32(np.mean(nll / np.maximum(target_lengths, 1)))


def _wrap_idx(lists, NIDX_):
    """lists: (NL, NIDX) int -> ap_gather wrapped layout [128, NIDX//16]."""
    out = np.zeros((128, NIDX_ // 16), np.int16)
    for g in range(NL):
        for j in range(NIDX_):
            out[g * 16 + (j % 16), j // 16] = lists[g, j]
    return out


# ----------------------------------------------------------------------------
# kernel builder
# ----------------------------------------------------------------------------

_NC_CACHE = {}


# fast-build bias: q = exp(x + FS) mirrors the proven-stable legacy
# q~ = exp(x + lnC + 0.5 - 1)/acc ~= exp(x - 1) since acc ~= C*e^0.5
FS = -1.0


def _build_fast(dims):
    """No-mask, renorm-free build: bf16 DP state on q = exp(x + FS)
    with NO on-device softmax division.  The per-(t,n) denominators
    factor out of the whole chain; their accumulators are shipped back
    and folded into the loss as sum_t ln(acc_t) on the host in f64.
    The chunk-feed path (gather -> small exp -> transpose) depends only
    on the px tile, never on the big exps or on VectorE."""
    T_, C_, L_ = dims
    S_, GW_, NIDX_, NPAIR_, TH_, _SHIFT_unused, _ = _derived(T_, C_, L_, False)

    import concourse.bacc as bacc
    import concourse.bass as bass
    import concourse.tile as tile
    from concourse import library_config, mybir

    f32 = mybir.dt.float32
    bf16 = mybir.dt.bfloat16
    Act = mybir.ActivationFunctionType

    HC = C_ // 2  # column-split half width

    nc = bacc.Bacc("TRN2", target_bir_lowering=False, debug=False)
    px = nc.dram_tensor("px", [T_ // TG, NL, TG, C_], f32,
                        kind="ExternalInput")
    idx_f = nc.dram_tensor("idx_f", [128, NIDX_ // 16], mybir.dt.int16,
                           kind="ExternalInput")
    idx_b = nc.dram_tensor("idx_b", [128, NIDX_ // 16], mybir.dt.int16,
                           kind="ExternalInput")
    res = nc.dram_tensor("res", [16, PAD + S_], f32, kind="ExternalOutput")
    resacc = nc.dram_tensor("resacc", [128, 4 * NPAIR_], f32,
                            kind="ExternalOutput")

    with tile.TileContext(nc) as tc:
        with (
            tc.tile_pool(name="main", bufs=5) as main_pool,
            tc.tile_pool(name="scr", bufs=1) as scr_pool,
            tc.tile_pool(name="qsb", bufs=4) as qsb_pool,
            tc.tile_pool(name="gout", bufs=4) as gout_pool,
            tc.tile_pool(name="qc", bufs=NPAIR_) as qc_pool,
            tc.tile_pool(name="single", bufs=1) as single,
        ):
            nc.gpsimd.load_library(library_config.ap_gather)

            ixf = single.tile([128, NIDX_ // 16], mybir.dt.int16, tag="ixf")
            ixb = single.tile([128, NIDX_ // 16], mybir.dt.int16, tag="ixb")
            nc.scalar.dma_start(out=ixf[:], in_=idx_f[:])
            nc.scalar.dma_start(out=ixb[:], in_=idx_b[:])

            shiftb = single.tile([128, 1], f32, tag="shiftb")
            nc.vector.memset(shiftb[:], FS)

            A = single.tile([16, PAD + S_], bf16, tag="A")
            t1 = single.tile([16, PAD + S_], bf16, tag="t1")
            nc.vector.memset(A[:], 0.0)
            nc.vector.memset(t1[:], 0.0)
            resf = single.tile([16, PAD + S_], f32, tag="resf")
            accall = single.tile([128, 4 * NPAIR_], f32, tag="accall")

            scr = scr_pool.tile([128, HC], f32, tag="scr")

            for j in range(NPAIR_):
                tg_f, tg_b = j, 2 * NPAIR_ - 1 - j
                qc = qc_pool.tile([16, TG, S_], bf16, tag="qc")

                for dr, tg in ((0, tg_f), (1, tg_b)):
                    mt = main_pool.tile([128, C_], f32, tag="mt")
                    nc.sync.dma_start(
                        out=mt[:],
                        in_=px[tg].rearrange("n t c -> (n t) c"))
                    # chunk-feed path first (scalar queue order): the
                    # small exp waits only on the gather, not the big exps
                    go = gout_pool.tile([128, NIDX_], f32, tag="go")
                    ix = ixf if dr == 0 else ixb
                    nc.gpsimd.ap_gather(go[:], mt[:], ix[:],
                                        channels=128, num_elems=C_, d=1,
                                        num_idxs=NIDX_)
                    qs = qsb_pool.tile([128, S_], bf16, tag="qs")
                    nc.scalar.activation(qs[:], go[:, 0:S_], Act.Exp,
                                         bias=shiftb[:, 0:1], scale=1.0)
                    # SBUF->SBUF transpose into the chunk tile, split
                    # across the gpsimd and scalar DGE queues (sync stays
                    # dedicated to the px stream: a qs-gated trigger there
                    # would stall the input DMA pipeline)
                    qcap = qc[:]
                    for eng, n0, nn in ((nc.gpsimd, 0, 4),
                                        (nc.scalar, 4, 4)):
                        if dr == 0:
                            dofs = n0 * TG * S_
                            tstep = S_
                        else:
                            dofs = (NL + n0) * TG * S_ + (TG - 1) * S_
                            tstep = -S_
                        dst = bass.AP(
                            tensor=qcap.tensor,
                            offset=qcap.offset + dofs,
                            ap=[[TG * S_, nn], [tstep, TG], [1, S_]],
                        )
                        src = bass.AP(
                            tensor=qs[:].tensor,
                            offset=qs[:].offset + n0 * TG * S_,
                            ap=[[S_, nn * TG], [1, S_]],
                        )
                        eng.dma_start(out=dst, in_=src)
                    # softmax denominators (host-corrected): accumulate
                    # row sums of exp into accall, two column halves;
                    # emitted last so the scalar queue serves the
                    # chunk-feed path (small exp + transpose) first
                    cb = j * 4 + dr * 2
                    nc.scalar.activation(scr[:], mt[:, 0:HC], Act.Exp,
                                         bias=0.0, scale=1.0,
                                         accum_out=accall[:, cb:cb + 1])
                    nc.scalar.activation(scr[:], mt[:, HC:C_], Act.Exp,
                                         bias=0.0, scale=1.0,
                                         accum_out=accall[:, cb + 1:cb + 2])

                for k16 in range(TG):
                    k = j * TG + k16
                    qk = qc[:, k16, 0:S_]
                    if k == 0:
                        nc.vector.tensor_scalar_add(
                            A[:, PAD:PAD + 2], qc[:, 0, 0:2], 0.0)
                        continue
                    nc.vector.tensor_add(t1[:, PAD:], A[:, PAD:],
                                         A[:, PAD - 1:PAD + S_ - 1])
                    dst_odd = t1[:, PAD + 1:PAD + S_].rearrange(
                        "p (a b) -> p a b", b=2)[:, :, 0]
                    src_odd = A[:, PAD - 1:PAD + S_ - 2].rearrange(
                        "p (a b) -> p a b", b=2)[:, :, 0]
                    nc.vector.tensor_add(dst_odd, dst_odd, src_odd)
                    nc.vector.tensor_mul(A[:, PAD:], t1[:, PAD:], qk)

            nc.scalar.dma_start(out=resacc[:], in_=accall[:])
            nc.scalar.activation(resf[:], A[:], Act.Copy)
            nc.sync.dma_start(out=res[:], in_=resf[:])
    nc.compile()
    return nc


def _build(use_masks, use_renorm, dims):
    T_, C_, L_ = dims
    S_, GW_, NIDX_, NPAIR_, TH_, SHIFT_, renorm_steps = _derived(
        T_, C_, L_, use_renorm)

    import concourse.bacc as bacc
    import concourse.bass as bass
    import concourse.tile as tile
    from concourse import library_config, mybir

    f32 = mybir.dt.float32
    Alu = mybir.AluOpType
    Act = mybir.ActivationFunctionType

    nc = bacc.Bacc("TRN2", target_bir_lowering=False, debug=False)
    # shard pre-tiled on host: [tile-group, n, t16, c] so each [128, C] tile
    # load reads 128 consecutive 16KB rows (full HBM bandwidth)
    px = nc.dram_tensor("px", [T_ // TG, NL, TG, C_], f32,
                        kind="ExternalInput")
    idx_f = nc.dram_tensor("idx_f", [128, NIDX_ // 16], mybir.dt.int16,
                           kind="ExternalInput")
    idx_b = nc.dram_tensor("idx_b", [128, NIDX_ // 16], mybir.dt.int16,
                           kind="ExternalInput")
    if use_masks:
        maskd = nc.dram_tensor("maskd", [16, PAD + S_], f32,
                               kind="ExternalInput")
    res = nc.dram_tensor("res", [2, 16, PAD + S_], f32, kind="ExternalOutput")

    with tile.TileContext(nc) as tc:
        with (
            tc.tile_pool(name="main", bufs=3) as main_pool,
            tc.tile_pool(name="scr", bufs=1) as scr_pool,
            tc.tile_pool(name="qslab", bufs=4) as qslab_pool,
            tc.tile_pool(name="gout", bufs=4) as gout_pool,
            tc.tile_pool(name="qc", bufs=NPAIR_) as qc_pool,
            tc.tile_pool(name="sc", bufs=NPAIR_) as sc_pool,
            tc.tile_pool(name="single", bufs=1) as single,
        ):
            nc.gpsimd.load_library(library_config.ap_gather)

            ixf = single.tile([128, NIDX_ // 16], mybir.dt.int16, tag="ixf")
            ixb = single.tile([128, NIDX_ // 16], mybir.dt.int16, tag="ixb")
            nc.scalar.dma_start(out=ixf[:], in_=idx_f[:])
            nc.scalar.dma_start(out=ixb[:], in_=idx_b[:])
            if use_masks:
                msk = single.tile([16, PAD + S_], f32, tag="msk")
                nc.scalar.dma_start(out=msk[:], in_=maskd[:])

            shiftb = single.tile([128, 1], f32, tag="shiftb")
            nc.vector.memset(shiftb[:], SHIFT_)

            A = single.tile([16, PAD + S_], f32, tag="A")
            t1 = single.tile([16, PAD + S_], f32, tag="t1")
            nc.vector.memset(A[:], 0.0)
            nc.vector.memset(t1[:], 0.0)
            if use_masks:
                am = single.tile([16, PAD + S_], f32, tag="am")
                nc.vector.memset(am[:], 0.0)
            Rbuf = single.tile([16, 4], f32, tag="R")
            nc.vector.memset(Rbuf[:], 1.0)
            rinv = single.tile([16, 1], f32, tag="rinv")
            patch = single.tile([16, 1], f32, tag="patch")

            pending_renorm = False
            for j in range(NPAIR_):
                tg_f, tg_b = j, 2 * NPAIR_ - 1 - j
                if j == 0:
                    # first pair split in two so the DP-feeding chain starts
                    # as soon as the forward half lands
                    mtp = main_pool.tile([128, 2 * C_], f32, tag="mt")
                    nc.sync.dma_start(
                        out=mtp[:, 0:C_],
                        in_=px[tg_f].rearrange("n t c -> (n t) c"))
                    nc.sync.dma_start(
                        out=mtp[:, C_:2 * C_],
                        in_=px[tg_b].rearrange("n t c -> (n t) c"))
                    halves = [(0, mtp[:, 0:C_]), (1, mtp[:, C_:2 * C_])]
                else:
                    # one 4 MB DMA per pair: halves are the fwd and bwd
                    # groups, each 128 consecutive 16KB rows
                    mtp = main_pool.tile([128, 2 * C_], f32, tag="mt")
                    src = bass.AP(
                        tensor=px[:].tensor,
                        offset=px[:].offset + tg_f * 128 * C_,
                        ap=[[C_, 128], [(tg_b - tg_f) * 128 * C_, 2], [1, C_]],
                    )
                    dst = bass.AP(
                        tensor=mtp[:].tensor,
                        offset=mtp[:].offset,
                        ap=[[2 * C_, 128], [C_, 2], [1, C_]],
                    )
                    nc.sync.dma_start(out=dst, in_=src)
                    halves = [(0, mtp[:, 0:C_]), (1, mtp[:, C_:2 * C_])]

                # chunk tile first: both q slabs transpose straight into it
                qc = qc_pool.tile([16, TG, GW_], f32, tag="qc")

                for dr, mt in halves:
                    qs = qslab_pool.tile([128, GW_], f32, tag="qs")
                    scr = scr_pool.tile([128, C_], f32, tag="scr")
                    # exp + fused row-sum (softmax denominator -> col S)
                    nc.scalar.activation(scr[:], mt, Act.Exp,
                                         bias=0.0, scale=1.0,
                                         accum_out=qs[:, S_:S_ + 1])
                    go = gout_pool.tile([128, NIDX_], f32, tag="go")
                    ix = ixf if dr == 0 else ixb
                    nc.gpsimd.ap_gather(go[:], mt, ix[:],
                                        channels=128, num_elems=C_, d=1,
                                        num_idxs=NIDX_)
                    nc.scalar.activation(qs[:, 0:S_], go[:, 0:S_], Act.Exp,
                                         bias=shiftb[:, 0:1], scale=1.0)
                    # SBUF->SBUF transpose: q slab [(n,t16), w] -> chunk
                    # rows [n, t16 (reversed for bwd), w]; no DRAM bounce
                    qcap = qc[:]
                    if dr == 0:
                        dst = bass.AP(
                            tensor=qcap.tensor, offset=qcap.offset,
                            ap=[[TG * GW_, NL], [GW_, TG], [1, GW_]],
                        )
                    else:
                        dst = bass.AP(
                            tensor=qcap.tensor,
                            offset=(qcap.offset + NL * TG * GW_
                                    + (TG - 1) * GW_),
                            ap=[[TG * GW_, NL], [-GW_, TG], [1, GW_]],
                        )
                    nc.gpsimd.dma_start(out=dst, in_=qs[:])

                sc = sc_pool.tile([16, TG], f32, tag="sc")
                nc.vector.reciprocal(sc[:], qc[:, :, S_])

                for k16 in range(TG):
                    k = j * TG + k16
                    qk = qc[:, k16, 0:S_]
                    sck = sc[:, k16:k16 + 1]
                    if pending_renorm:
                        nc.vector.tensor_mul(patch[:], rinv[:], sck)
                        sck = patch[:, 0:1]
                        pending_renorm = False
                    if k == 0:
                        # A[s in {0,1}] = q * (1/acc), both chains
                        nc.vector.tensor_scalar_mul(
                            A[:, PAD:PAD + 2], qc[:, 0, 0:2], sck)
                        if use_masks:
                            nc.vector.tensor_mul(am[:, PAD:], A[:, PAD:],
                                                 msk[:, PAD:])
                        continue
                    # t1 = A + shift1(A)
                    nc.vector.tensor_add(t1[:, PAD:], A[:, PAD:],
                                         A[:, PAD - 1:PAD + S_ - 1])
                    if use_masks:
                        # t1 += shift2(masked A)
                        nc.vector.tensor_add(t1[:, PAD:], t1[:, PAD:],
                                             am[:, 0:S_])
                    else:
                        # odd states only: t1[s] += A[s-2]
                        dst_odd = t1[:, PAD + 1:PAD + S_].rearrange(
                            "p (a b) -> p a b", b=2)[:, :, 0]
                        src_odd = A[:, PAD - 1:PAD + S_ - 2].rearrange(
                            "p (a b) -> p a b", b=2)[:, :, 0]
                        nc.vector.tensor_add(dst_odd, dst_odd, src_odd)
                    # A' = (t1 * sc) * q
                    nc.vector.scalar_tensor_tensor(
                        A[:, PAD:], t1[:, PAD:], sck, qk,
                        op0=Alu.mult, op1=Alu.mult)
                    if use_masks:
                        nc.vector.tensor_mul(am[:, PAD:], A[:, PAD:],
                                             msk[:, PAD:])
                    if k in renorm_steps:
                        r = renorm_steps.index(k)
                        nc.vector.tensor_reduce(
                            Rbuf[:, r:r + 1], A[:, PAD:],
                            axis=mybir.AxisListType.X, op=Alu.max)
                        nc.vector.reciprocal(rinv[:], Rbuf[:, r:r + 1])
                        pending_renorm = True

            nc.sync.dma_start(out=res[0], in_=A[:])
            nc.sync.dma_start(out=res[1, :, 0:4], in_=Rbuf[:])
    nc.compile()
    return nc


def _get_nc(use_masks, use_renorm, dims):
    key = (use_masks, use_renorm, dims)
    if key not in _NC_CACHE:
        _NC_CACHE[key] = _build(use_masks, use_renorm, dims)
    return _NC_CACHE[key]


# ----------------------------------------------------------------------------
# device run for one full (T_, N=64, C_) problem
# ----------------------------------------------------------------------------

def _run_device(preds, t2d, dims, use_renorm):
    T_, C_, L_ = dims
    S_, GW_, NIDX_, NPAIR_, TH_, SHIFT_, renorm_steps = _derived(
        T_, C_, L_, use_renorm)

    ext = _ext_labels(t2d, S_)                    # (N, S)
    m_fwd = _skip_mask(ext)
    use_masks = bool((t2d[:, 1:] == t2d[:, :-1]).any())

    # m'[s] = m[s+2] (allowed s -> s+2); backward chain is state-reversed
    m_p = np.zeros_like(m_fwd)
    m_p[:, :-2] = m_fwd[:, 2:]
    m_bwd = m_p[:, ::-1]

    from concourse.bass_utils import run_bass_kernel_spmd

    fast = (not use_masks) and (not use_renorm)
    if fast:
        key = ("fast", dims)
        if key not in _NC_CACHE:
            _NC_CACHE[key] = _build_fast(dims)
        nc = _NC_CACHE[key]
    else:
        nc = _get_nc(use_masks, use_renorm, dims)

    in_maps = []
    for c in range(NCORES):
        n0 = c * NL
        # pre-tile: (T, NL, C) -> (T/TG, NL, TG, C) with (n, t16) row order
        shard = np.ascontiguousarray(
            preds[:, n0:n0 + NL, :]
            .reshape(T_ // TG, TG, NL, C_)
            .transpose(0, 2, 1, 3))
        lists_f = np.zeros((NL, NIDX_), np.int64)
        lists_b = np.zeros((NL, NIDX_), np.int64)
        lists_f[:, :S_] = ext[n0:n0 + NL]
        lists_b[:, :S_] = ext[n0:n0 + NL, ::-1]
        im = {
            "px": shard,
            "idx_f": _wrap_idx(lists_f, NIDX_),
            "idx_b": _wrap_idx(lists_b, NIDX_),
        }
        if use_masks:
            # am-premask: am[x] = A[x] * M[x+2] so that am[s-2] carries the
            # destination mask M[s]
            mam_f = np.zeros_like(m_fwd)
            mam_f[:, :-2] = m_fwd[:, 2:]
            mam_b = np.zeros_like(m_bwd)
            mam_b[:, :-2] = m_bwd[:, 2:]
            mtile = np.zeros((16, PAD + S_), np.float32)
            mtile[0:NL, PAD:] = mam_f[n0:n0 + NL]
            mtile[NL:16, PAD:] = mam_b[n0:n0 + NL]
            im["maskd"] = mtile
        in_maps.append(im)

    out = run_bass_kernel_spmd(nc, in_maps, core_ids=list(range(NCORES)))

    # host stitch (float64): combine the two chains at the midpoint
    nr = 0 if fast else len(renorm_steps)
    losses = np.zeros(NCORES * NL, np.float64)
    for c in range(NCORES):
        resv = out.results[c]["res"].astype(np.float64)
        if fast:
            resv = resv[None]              # [1, 16, PAD+S] like res[0]
            # resacc rows are (n, t16) within tile-group g; column block
            # j*4 + dr*2 holds the two col-half row sums of exp(preds)
            racc = out.results[c]["resacc"].astype(np.float64)
            acc = racc[:, 0::2] + racc[:, 1::2]      # [128, 2*NPAIR]
            # column 2j+dr -> tile-group tg (fwd: tg=j, bwd: tg=15-j)
            lnacc = np.zeros(NL)                      # sum_t ln acc per n
            for col in range(acc.shape[1]):
                jj, dr = col // 2, col % 2
                # all T slices contribute regardless of tg identity
                la = np.log(acc[:, col]).reshape(NL, TG)
                lnacc += la.sum(axis=1)
        for n in range(NL):
            gn = c * NL + n
            a = resv[0, n, PAD:]           # alpha_{TH-1}, natural s order
            b = resv[0, NL + n, PAD:]      # beta_{TH}, reversed s order
            mb = m_bwd[gn]
            be = b.copy()
            be[1:] += b[:-1]
            be[2:] += np.where(mb[2:], b[:-2], 0.0)
            v = float((a[::-1] * be).sum())
            if fast:
                ll = np.log(v) - lnacc[n] - T_ * FS
            else:
                rln = 0.0
                if nr:
                    rln = (np.log(resv[1, n, 0:nr]).sum()
                           + np.log(resv[1, NL + n, 0:nr]).sum())
                ll = np.log(v) + rln - T_ * SHIFT_
            losses[gn] = -ll / L_
    return np.float32(losses.mean())


# ----------------------------------------------------------------------------
# entry point
# ----------------------------------------------------------------------------

def kernel(preds, targets, pred_lengths, target_lengths):
    preds = np.asarray(preds, np.float32)
    targets = np.asarray(targets, np.int32)
    pred_lengths = np.asarray(pred_lengths, np.int32)
    target_lengths = np.asarray(target_lengths, np.int32)
    t2d = targets.reshape(N, L)

    fast_ok = (
        preds.shape == (T, N, C)
        and targets.shape == (N * L,)
        and np.all(pred_lengths == T)
        and np.all(target_lengths == L)
        and np.all(targets >= 1)
        and np.all(targets < C)
        and np.isfinite(preds).all()
        and np.abs(preds).max() < 60.0
    )
    if not fast_ok:
        return _ref_numpy(preds, t2d, pred_lengths, target_lengths)

    # renorm-free DP is exact and in-range for modest logits; the
    # renormalising build remains the guard for unusual magnitudes
    use_renorm = bool(np.abs(preds).max() >= 8.0)
    return _run_device(preds, t2d, (T, C, L), use_renorm)



# revision 17
# speedup vs baseline: 1.1869x; 1.1869x over previous
"""CTC loss (nn_CTCLoss) Trainium2 Bass kernel.

Sharding: data-parallel over batch N across 8 cores (8 samples/core).

Per core:
  * Bulk pass: stream the (T, NL=8, C) f32 shard through SBUF as
    [128=(n,t16), C] tiles.  ScalarE computes exp() with a fused per-row
    accumulate (softmax denominator per (t, n)); GpSimd ap_gather pulls
    the S=2L+1 extended-label columns per sample (index lists are
    host-prepared; each 16-partition group shares one sample's list);
    ScalarE exponentiates the gathered logits (bias=+SHIFT).  q rows plus
    the accumulator column bounce through a DRAM scratch to move t from
    partitions onto the free axis in DP order (the backward direction is
    stored time-reversed and gathered state-reversed, so both chains read
    forward).
  * DP pass on VectorE in the probability domain: forward chain on
    partitions 0-7, time/state-reversed backward chain on partitions
    8-15, advanced together by 3 (no label repeats) or 4 (general)
    instructions per time step; T/2 sequential steps per chain.  The
    softmax denominator is folded in per step via the scalar slot of
    scalar_tensor_tensor (scalar = 1/acc, periodically also carrying a
    renormalisation factor).
  * Final alpha/beta states plus renorm maxima (16 x ~70 floats/core) go
    back to the host, which stitches the chains at the midpoint in
    float64 and takes the batch mean.
"""

import sys

import numpy as np

for _p in ("/root/.axon_site", "/root/.axon_site/_ro/trn_rl_repo", "/opt/trn_rl_repo"):
    if _p not in sys.path:
        sys.path.append(_p)

NCORES = 8
NL = 8                   # samples per core
TG = 16                  # time steps per tile group (128 = NL * TG partitions)
BLANK = 0
PAD = 2                  # leading zero pad columns in DP tiles

# problem dims (the graded configuration)
T, N, C, L = 256, 64, 4096, 32


def _derived(T_, C_, L_, use_renorm=True):
    S_ = 2 * L_ + 1
    GW_ = S_ + 1                       # q row + acc column in DRAM scratch
    NIDX_ = ((S_ + 15) // 16) * 16     # gather list length (%16 == 0)
    NPAIR_ = T_ // (2 * TG)
    TH_ = T_ // 2                      # steps per chain
    if use_renorm:
        SHIFT_ = float(np.log(C_) + 0.5)
        renorm_ = tuple(range(31, TH_ - 1, 32))
    else:
        # -1.0 cancels the ~e^1 per-step growth of the summed mass; the
        # chain then stays inside fp32 range with no renorms at all
        SHIFT_ = float(np.log(C_) + 0.5) - 1.0
        renorm_ = ()
    return S_, GW_, NIDX_, NPAIR_, TH_, SHIFT_, renorm_


# ----------------------------------------------------------------------------
# host-side helpers
# ----------------------------------------------------------------------------

def _ext_labels(t2d, S_):
    ext = np.zeros((t2d.shape[0], S_), np.int64)
    ext[:, 1::2] = t2d
    return ext


def _skip_mask(ext):
    sidx = np.arange(ext.shape[1])
    return (
        (sidx[None, :] >= 2)
        & (ext != BLANK)
        & (ext != np.roll(ext, 2, axis=1))
    )


def _ref_numpy(preds, t2d, pred_lengths, target_lengths):
    """float64 port of the reference (fallback path)."""
    preds = preds.astype(np.float64)
    Tn, n = preds.shape[0], preds.shape[1]
    S_ = 2 * t2d.shape[1] + 1
    mx = preds.max(axis=2, keepdims=True)
    lp = preds - mx - np.log(np.exp(preds - mx).sum(axis=2, keepdims=True))
    ext = _ext_labels(t2d, S_)
    lpe = lp[:, np.arange(n)[:, None], ext]
    skip_ok = _skip_mask(ext)
    NEGI = -1e30
    sidx = np.arange(S_)
    valid = sidx[None, :] < (2 * target_lengths[:, None] + 1)
    alpha = np.full((n, S_), NEGI)
    alpha[:, 0] = lpe[0, :, 0]
    alpha[:, 1] = np.where(target_lengths > 0, lpe[0, :, 1], NEGI)
    alpha = np.where(valid, alpha, NEGI)

    def lse(*a):
        m = np.maximum.reduce(a)
        m = np.where(np.isfinite(m), m, 0.0)
        return m + np.log(sum(np.exp(x - m) for x in a))

    for t in range(1, Tn):
        a2 = np.concatenate([np.full((n, 1), NEGI), alpha[:, :-1]], 1)
        a3 = np.concatenate([np.full((n, 2), NEGI), alpha[:, :-2]], 1)
        a3 = np.where(skip_ok, a3, NEGI)
        new = np.where(valid, lse(alpha, a2, a3) + lpe[t], NEGI)
        active = (t < pred_lengths)[:, None]
        alpha = np.where(active, new, alpha)
    end = 2 * target_lengths
    a_last = alpha[np.arange(n), end]
    a_prev = alpha[np.arange(n), np.maximum(end - 1, 0)]
    a_prev = np.where(target_lengths > 0, a_prev, NEGI)
    nll = -lse(a_last, a_prev)
    nll = np.where(np.isfinite(nll) & (nll < 1e29), nll, 0.0)
    return np.float32(np.mean(nll / np.maximum(target_lengths, 1)))


def _wrap_idx(lists, NIDX_):
    """lists: (NL, NIDX) int -> ap_gather wrapped layout [128, NIDX//16]."""
    out = np.zeros((128, NIDX_ // 16), np.int16)
    for g in range(NL):
        for j in range(NIDX_):
            out[g * 16 + (j % 16), j // 16] = lists[g, j]
    return out


# ----------------------------------------------------------------------------
# kernel builder
# ----------------------------------------------------------------------------

_NC_CACHE = {}


# fast-build bias: q = exp(x + FS) mirrors the proven-stable legacy
# q~ = exp(x + lnC + 0.5 - 1)/acc ~= exp(x - 1) since acc ~= C*e^0.5
FS = -1.0


def _build_fast(dims):
    """No-mask, renorm-free build: bf16 DP state on q = exp(x + FS)
    with NO on-device softmax division.  The per-(t,n) denominators
    factor out of the whole chain; their accumulators are shipped back
    and folded into the loss as sum_t ln(acc_t) on the host in f64.
    The chunk-feed path (gather -> small exp -> transpose) depends only
    on the px tile, never on the big exps or on VectorE."""
    T_, C_, L_ = dims
    S_, GW_, NIDX_, NPAIR_, TH_, _SHIFT_unused, _ = _derived(T_, C_, L_, False)

    import concourse.bacc as bacc
    import concourse.bass as bass
    import concourse.tile as tile
    from concourse import library_config, mybir

    f32 = mybir.dt.float32
    bf16 = mybir.dt.bfloat16
    Act = mybir.ActivationFunctionType

    HC = C_ // 2  # column-split half width

    nc = bacc.Bacc("TRN2", target_bir_lowering=False, debug=False)
    px = nc.dram_tensor("px", [T_ // TG, NL, TG, C_], f32,
                        kind="ExternalInput")
    idx_f = nc.dram_tensor("idx_f", [128, NIDX_ // 16], mybir.dt.int16,
                           kind="ExternalInput")
    idx_b = nc.dram_tensor("idx_b", [128, NIDX_ // 16], mybir.dt.int16,
                           kind="ExternalInput")
    res = nc.dram_tensor("res", [16, PAD + S_], f32, kind="ExternalOutput")
    resacc = nc.dram_tensor("resacc", [128, 4 * NPAIR_], f32,
                            kind="ExternalOutput")

    with tile.TileContext(nc) as tc:
        with (
            tc.tile_pool(name="main", bufs=5) as main_pool,
            tc.tile_pool(name="scr", bufs=1) as scr_pool,
            tc.tile_pool(name="qsb", bufs=4) as qsb_pool,
            tc.tile_pool(name="gout", bufs=4) as gout_pool,
            tc.tile_pool(name="qc", bufs=NPAIR_) as qc_pool,
            tc.tile_pool(name="single", bufs=1) as single,
        ):
            nc.gpsimd.load_library(library_config.ap_gather)

            ixf = single.tile([128, NIDX_ // 16], mybir.dt.int16, tag="ixf")
            ixb = single.tile([128, NIDX_ // 16], mybir.dt.int16, tag="ixb")
            nc.scalar.dma_start(out=ixf[:], in_=idx_f[:])
            nc.scalar.dma_start(out=ixb[:], in_=idx_b[:])

            shiftb = single.tile([128, 1], f32, tag="shiftb")
            nc.vector.memset(shiftb[:], FS)

            A = single.tile([16, PAD + S_], bf16, tag="A")
            t1 = single.tile([16, PAD + S_], bf16, tag="t1")
            nc.vector.memset(A[:], 0.0)
            nc.vector.memset(t1[:], 0.0)
            resf = single.tile([16, PAD + S_], f32, tag="resf")
            accall = single.tile([128, 4 * NPAIR_], f32, tag="accall")

            scr = scr_pool.tile([128, HC], f32, tag="scr")

            for j in range(NPAIR_):
                tg_f, tg_b = j, 2 * NPAIR_ - 1 - j
                qc = qc_pool.tile([16, TG, S_], bf16, tag="qc")

                for dr, tg in ((0, tg_f), (1, tg_b)):
                    mt = main_pool.tile([128, C_], f32, tag="mt")
                    nc.sync.dma_start(
                        out=mt[:],
                        in_=px[tg].rearrange("n t c -> (n t) c"))
                    # chunk-feed path first (scalar queue order): the
                    # small exp waits only on the gather, not the big exps
                    go = gout_pool.tile([128, NIDX_], f32, tag="go")
                    ix = ixf if dr == 0 else ixb
                    nc.gpsimd.ap_gather(go[:], mt[:], ix[:],
                                        channels=128, num_elems=C_, d=1,
                                        num_idxs=NIDX_)
                    qs = qsb_pool.tile([128, S_], bf16, tag="qs")
                    nc.scalar.activation(qs[:], go[:, 0:S_], Act.Exp,
                                         bias=shiftb[:, 0:1], scale=1.0)
                    # SBUF->SBUF transpose into the chunk tile (gpsimd
                    # DGE queue; split attempts across other engine
                    # queues measured slower)
                    qcap = qc[:]
                    if dr == 0:
                        dst = bass.AP(
                            tensor=qcap.tensor, offset=qcap.offset,
                            ap=[[TG * S_, NL], [S_, TG], [1, S_]],
                        )
                    else:
                        dst = bass.AP(
                            tensor=qcap.tensor,
                            offset=(qcap.offset + NL * TG * S_
                                    + (TG - 1) * S_),
                            ap=[[TG * S_, NL], [-S_, TG], [1, S_]],
                        )
                    nc.gpsimd.dma_start(out=dst, in_=qs[:])
                    # softmax denominators (host-corrected): accumulate
                    # row sums of exp into accall, two column halves;
                    # emitted last so the scalar queue serves the
                    # chunk-feed path (small exp + transpose) first
                    cb = j * 4 + dr * 2
                    nc.scalar.activation(scr[:], mt[:, 0:HC], Act.Exp,
                                         bias=0.0, scale=1.0,
                                         accum_out=accall[:, cb:cb + 1])
                    nc.scalar.activation(scr[:], mt[:, HC:C_], Act.Exp,
                                         bias=0.0, scale=1.0,
                                         accum_out=accall[:, cb + 1:cb + 2])

                for k16 in range(TG):
                    k = j * TG + k16
                    qk = qc[:, k16, 0:S_]
                    if k == 0:
                        nc.vector.tensor_scalar_add(
                            A[:, PAD:PAD + 2], qc[:, 0, 0:2], 0.0)
                        continue
                    nc.vector.tensor_add(t1[:, PAD:], A[:, PAD:],
                                         A[:, PAD - 1:PAD + S_ - 1])
                    dst_odd = t1[:, PAD + 1:PAD + S_].rearrange(
                        "p (a b) -> p a b", b=2)[:, :, 0]
                    src_odd = A[:, PAD - 1:PAD + S_ - 2].rearrange(
                        "p (a b) -> p a b", b=2)[:, :, 0]
                    nc.vector.tensor_add(dst_odd, dst_odd, src_odd)
                    nc.vector.tensor_mul(A[:, PAD:], t1[:, PAD:], qk)

            nc.scalar.dma_start(out=resacc[:], in_=accall[:])
            nc.scalar.activation(resf[:], A[:], Act.Copy)
            nc.sync.dma_start(out=res[:], in_=resf[:])
    nc.compile()
    return nc


def _build(use_masks, use_renorm, dims):
    T_, C_, L_ = dims
    S_, GW_, NIDX_, NPAIR_, TH_, SHIFT_, renorm_steps = _derived(
        T_, C_, L_, use_renorm)

    import concourse.bacc as bacc
    import concourse.bass as bass
    import concourse.tile as tile
    from concourse import library_config, mybir

    f32 = mybir.dt.float32
    Alu = mybir.AluOpType
    Act = mybir.ActivationFunctionType

    nc = bacc.Bacc("TRN2", target_bir_lowering=False, debug=False)
    # shard pre-tiled on host: [tile-group, n, t16, c] so each [128, C] tile
    # load reads 128 consecutive 16KB rows (full HBM bandwidth)
    px = nc.dram_tensor("px", [T_ // TG, NL, TG, C_], f32,
                        kind="ExternalInput")
    idx_f = nc.dram_tensor("idx_f", [128, NIDX_ // 16], mybir.dt.int16,
                           kind="ExternalInput")
    idx_b = nc.dram_tensor("idx_b", [128, NIDX_ // 16], mybir.dt.int16,
                           kind="ExternalInput")
    if use_masks:
        maskd = nc.dram_tensor("maskd", [16, PAD + S_], f32,
                               kind="ExternalInput")
    res = nc.dram_tensor("res", [2, 16, PAD + S_], f32, kind="ExternalOutput")

    with tile.TileContext(nc) as tc:
        with (
            tc.tile_pool(name="main", bufs=3) as main_pool,
            tc.tile_pool(name="scr", bufs=1) as scr_pool,
            tc.tile_pool(name="qslab", bufs=4) as qslab_pool,
            tc.tile_pool(name="gout", bufs=4) as gout_pool,
            tc.tile_pool(name="qc", bufs=NPAIR_) as qc_pool,
            tc.tile_pool(name="sc", bufs=NPAIR_) as sc_pool,
            tc.tile_pool(name="single", bufs=1) as single,
        ):
            nc.gpsimd.load_library(library_config.ap_gather)

            ixf = single.tile([128, NIDX_ // 16], mybir.dt.int16, tag="ixf")
            ixb = single.tile([128, NIDX_ // 16], mybir.dt.int16, tag="ixb")
            nc.scalar.dma_start(out=ixf[:], in_=idx_f[:])
            nc.scalar.dma_start(out=ixb[:], in_=idx_b[:])
            if use_masks:
                msk = single.tile([16, PAD + S_], f32, tag="msk")
                nc.scalar.dma_start(out=msk[:], in_=maskd[:])

            shiftb = single.tile([128, 1], f32, tag="shiftb")
            nc.vector.memset(shiftb[:], SHIFT_)

            A = single.tile([16, PAD + S_], f32, tag="A")
            t1 = single.tile([16, PAD + S_], f32, tag="t1")
            nc.vector.memset(A[:], 0.0)
            nc.vector.memset(t1[:], 0.0)
            if use_masks:
                am = single.tile([16, PAD + S_], f32, tag="am")
                nc.vector.memset(am[:], 0.0)
            Rbuf = single.tile([16, 4], f32, tag="R")
            nc.vector.memset(Rbuf[:], 1.0)
            rinv = single.tile([16, 1], f32, tag="rinv")
            patch = single.tile([16, 1], f32, tag="patch")

            pending_renorm = False
            for j in range(NPAIR_):
                tg_f, tg_b = j, 2 * NPAIR_ - 1 - j
                if j == 0:
                    # first pair split in two so the DP-feeding chain starts
                    # as soon as the forward half lands
                    mtp = main_pool.tile([128, 2 * C_], f32, tag="mt")
                    nc.sync.dma_start(
                        out=mtp[:, 0:C_],
                        in_=px[tg_f].rearrange("n t c -> (n t) c"))
                    nc.sync.dma_start(
                        out=mtp[:, C_:2 * C_],
                        in_=px[tg_b].rearrange("n t c -> (n t) c"))
                    halves = [(0, mtp[:, 0:C_]), (1, mtp[:, C_:2 * C_])]
                else:
                    # one 4 MB DMA per pair: halves are the fwd and bwd
                    # groups, each 128 consecutive 16KB rows
                    mtp = main_pool.tile([128, 2 * C_], f32, tag="mt")
                    src = bass.AP(
                        tensor=px[:].tensor,
                        offset=px[:].offset + tg_f * 128 * C_,
                        ap=[[C_, 128], [(tg_b - tg_f) * 128 * C_, 2], [1, C_]],
                    )
                    dst = bass.AP(
                        tensor=mtp[:].tensor,
                        offset=mtp[:].offset,
                        ap=[[2 * C_, 128], [C_, 2], [1, C_]],
                    )
                    nc.sync.dma_start(out=dst, in_=src)
                    halves = [(0, mtp[:, 0:C_]), (1, mtp[:, C_:2 * C_])]

                # chunk tile first: both q slabs transpose straight into it
                qc = qc_pool.tile([16, TG, GW_], f32, tag="qc")

                for dr, mt in halves:
                    qs = qslab_pool.tile([128, GW_], f32, tag="qs")
                    scr = scr_pool.tile([128, C_], f32, tag="scr")
                    # exp + fused row-sum (softmax denominator -> col S)
                    nc.scalar.activation(scr[:], mt, Act.Exp,
                                         bias=0.0, scale=1.0,
                                         accum_out=qs[:, S_:S_ + 1])
                    go = gout_pool.tile([128, NIDX_], f32, tag="go")
                    ix = ixf if dr == 0 else ixb
                    nc.gpsimd.ap_gather(go[:], mt, ix[:],
                                        channels=128, num_elems=C_, d=1,
                                        num_idxs=NIDX_)
                    nc.scalar.activation(qs[:, 0:S_], go[:, 0:S_], Act.Exp,
                                         bias=shiftb[:, 0:1], scale=1.0)
                    # SBUF->SBUF transpose: q slab [(n,t16), w] -> chunk
                    # rows [n, t16 (reversed for bwd), w]; no DRAM bounce
                    qcap = qc[:]
                    if dr == 0:
                        dst = bass.AP(
                            tensor=qcap.tensor, offset=qcap.offset,
                            ap=[[TG * GW_, NL], [GW_, TG], [1, GW_]],
                        )
                    else:
                        dst = bass.AP(
                            tensor=qcap.tensor,
                            offset=(qcap.offset + NL * TG * GW_
                                    + (TG - 1) * GW_),
                            ap=[[TG * GW_, NL], [-GW_, TG], [1, GW_]],
                        )
                    nc.gpsimd.dma_start(out=dst, in_=qs[:])

                sc = sc_pool.tile([16, TG], f32, tag="sc")
                nc.vector.reciprocal(sc[:], qc[:, :, S_])

                for k16 in range(TG):
                    k = j * TG + k16
                    qk = qc[:, k16, 0:S_]
                    sck = sc[:, k16:k16 + 1]
                    if pending_renorm:
                        nc.vector.tensor_mul(patch[:], rinv[:], sck)
                        sck = patch[:, 0:1]
                        pending_renorm = False
                    if k == 0:
                        # A[s in {0,1}] = q * (1/acc), both chains
                        nc.vector.tensor_scalar_mul(
                            A[:, PAD:PAD + 2], qc[:, 0, 0:2], sck)
                        if use_masks:
                            nc.vector.tensor_mul(am[:, PAD:], A[:, PAD:],
                                                 msk[:, PAD:])
                        continue
                    # t1 = A + shift1(A)
                    nc.vector.tensor_add(t1[:, PAD:], A[:, PAD:],
                                         A[:, PAD - 1:PAD + S_ - 1])
                    if use_masks:
                        # t1 += shift2(masked A)
                        nc.vector.tensor_add(t1[:, PAD:], t1[:, PAD:],
                                             am[:, 0:S_])
                    else:
                        # odd states only: t1[s] += A[s-2]
                        dst_odd = t1[:, PAD + 1:PAD + S_].rearrange(
                            "p (a b) -> p a b", b=2)[:, :, 0]
                        src_odd = A[:, PAD - 1:PAD + S_ - 2].rearrange(
                            "p (a b) -> p a b", b=2)[:, :, 0]
                        nc.vector.tensor_add(dst_odd, dst_odd, src_odd)
                    # A' = (t1 * sc) * q
                    nc.vector.scalar_tensor_tensor(
                        A[:, PAD:], t1[:, PAD:], sck, qk,
                        op0=Alu.mult, op1=Alu.mult)
                    if use_masks:
                        nc.vector.tensor_mul(am[:, PAD:], A[:, PAD:],
                                             msk[:, PAD:])
                    if k in renorm_steps:
                        r = renorm_steps.index(k)
                        nc.vector.tensor_reduce(
                            Rbuf[:, r:r + 1], A[:, PAD:],
                            axis=mybir.AxisListType.X, op=Alu.max)
                        nc.vector.reciprocal(rinv[:], Rbuf[:, r:r + 1])
                        pending_renorm = True

            nc.sync.dma_start(out=res[0], in_=A[:])
            nc.sync.dma_start(out=res[1, :, 0:4], in_=Rbuf[:])
    nc.compile()
    return nc


def _get_nc(use_masks, use_renorm, dims):
    key = (use_masks, use_renorm, dims)
    if key not in _NC_CACHE:
        _NC_CACHE[key] = _build(use_masks, use_renorm, dims)
    return _NC_CACHE[key]


# ----------------------------------------------------------------------------
# device run for one full (T_, N=64, C_) problem
# ----------------------------------------------------------------------------

def _run_device(preds, t2d, dims, use_renorm):
    T_, C_, L_ = dims
    S_, GW_, NIDX_, NPAIR_, TH_, SHIFT_, renorm_steps = _derived(
        T_, C_, L_, use_renorm)

    ext = _ext_labels(t2d, S_)                    # (N, S)
    m_fwd = _skip_mask(ext)
    use_masks = bool((t2d[:, 1:] == t2d[:, :-1]).any())

    # m'[s] = m[s+2] (allowed s -> s+2); backward chain is state-reversed
    m_p = np.zeros_like(m_fwd)
    m_p[:, :-2] = m_fwd[:, 2:]
    m_bwd = m_p[:, ::-1]

    from concourse.bass_utils import run_bass_kernel_spmd

    fast = (not use_masks) and (not use_renorm)
    if fast:
        key = ("fast", dims)
        if key not in _NC_CACHE:
            _NC_CACHE[key] = _build_fast(dims)
        nc = _NC_CACHE[key]
    else:
        nc = _get_nc(use_masks, use_renorm, dims)

    in_maps = []
    for c in range(NCORES):
        n0 = c * NL
        # pre-tile: (T, NL, C) -> (T/TG, NL, TG, C) with (n, t16) row order
        shard = np.ascontiguousarray(
            preds[:, n0:n0 + NL, :]
            .reshape(T_ // TG, TG, NL, C_)
            .transpose(0, 2, 1, 3))
        lists_f = np.zeros((NL, NIDX_), np.int64)
        lists_b = np.zeros((NL, NIDX_), np.int64)
        lists_f[:, :S_] = ext[n0:n0 + NL]
        lists_b[:, :S_] = ext[n0:n0 + NL, ::-1]
        im = {
            "px": shard,
            "idx_f": _wrap_idx(lists_f, NIDX_),
            "idx_b": _wrap_idx(lists_b, NIDX_),
        }
        if use_masks:
            # am-premask: am[x] = A[x] * M[x+2] so that am[s-2] carries the
            # destination mask M[s]
            mam_f = np.zeros_like(m_fwd)
            mam_f[:, :-2] = m_fwd[:, 2:]
            mam_b = np.zeros_like(m_bwd)
            mam_b[:, :-2] = m_bwd[:, 2:]
            mtile = np.zeros((16, PAD + S_), np.float32)
            mtile[0:NL, PAD:] = mam_f[n0:n0 + NL]
            mtile[NL:16, PAD:] = mam_b[n0:n0 + NL]
            im["maskd"] = mtile
        in_maps.append(im)

    out = run_bass_kernel_spmd(nc, in_maps, core_ids=list(range(NCORES)))

    # host stitch (float64): combine the two chains at the midpoint
    nr = 0 if fast else len(renorm_steps)
    losses = np.zeros(NCORES * NL, np.float64)
    for c in range(NCORES):
        resv = out.results[c]["res"].astype(np.float64)
        if fast:
            resv = resv[None]              # [1, 16, PAD+S] like res[0]
            # resacc rows are (n, t16) within tile-group g; column block
            # j*4 + dr*2 holds the two col-half row sums of exp(preds)
            racc = out.results[c]["resacc"].astype(np.float64)
            acc = racc[:, 0::2] + racc[:, 1::2]      # [128, 2*NPAIR]
            # column 2j+dr -> tile-group tg (fwd: tg=j, bwd: tg=15-j)
            lnacc = np.zeros(NL)                      # sum_t ln acc per n
            for col in range(acc.shape[1]):
                jj, dr = col // 2, col % 2
                # all T slices contribute regardless of tg identity
                la = np.log(acc[:, col]).reshape(NL, TG)
                lnacc += la.sum(axis=1)
        for n in range(NL):
            gn = c * NL + n
            a = resv[0, n, PAD:]           # alpha_{TH-1}, natural s order
            b = resv[0, NL + n, PAD:]      # beta_{TH}, reversed s order
            mb = m_bwd[gn]
            be = b.copy()
            be[1:] += b[:-1]
            be[2:] += np.where(mb[2:], b[:-2], 0.0)
            v = float((a[::-1] * be).sum())
            if fast:
                ll = np.log(v) - lnacc[n] - T_ * FS
            else:
                rln = 0.0
                if nr:
                    rln = (np.log(resv[1, n, 0:nr]).sum()
                           + np.log(resv[1, NL + n, 0:nr]).sum())
                ll = np.log(v) + rln - T_ * SHIFT_
            losses[gn] = -ll / L_
    return np.float32(losses.mean())


# ----------------------------------------------------------------------------
# entry point
# ----------------------------------------------------------------------------

def kernel(preds, targets, pred_lengths, target_lengths):
    preds = np.asarray(preds, np.float32)
    targets = np.asarray(targets, np.int32)
    pred_lengths = np.asarray(pred_lengths, np.int32)
    target_lengths = np.asarray(target_lengths, np.int32)
    t2d = targets.reshape(N, L)

    fast_ok = (
        preds.shape == (T, N, C)
        and targets.shape == (N * L,)
        and np.all(pred_lengths == T)
        and np.all(target_lengths == L)
        and np.all(targets >= 1)
        and np.all(targets < C)
        and np.isfinite(preds).all()
        and np.abs(preds).max() < 60.0
    )
    if not fast_ok:
        return _ref_numpy(preds, t2d, pred_lengths, target_lengths)

    # renorm-free DP is exact and in-range for modest logits; the
    # renormalising build remains the guard for unusual magnitudes
    use_renorm = bool(np.abs(preds).max() >= 8.0)
    return _run_device(preds, t2d, (T, C, L), use_renorm)

